# revision 1
# baseline (speedup 1.0000x reference)
"""Trainium2 Bass kernel for nn_Decoder: 2-layer GRU decoder, batch-parallel over 8 cores.

Strategy:
  - Shard batch 128 -> 16 rows/core, replicate all weights (fp16, SBUF-resident).
  - Recurrence (256 steps): transposed activations (hidden on partitions) are the
    stationary matmul operand; weight slices stream through the PE as the moving
    operand.  PSUM accumulates gi+gh for the r/z gates directly.
  - fc computed twice per step: transposed (weights stationary) so the sigmoid
    output is directly the next step's stationary operand xT, and batch-major for
    the output row DMA.
  - f32 master hidden state; fp16 only at matmul operands.
  - glob @ W_ih0[:, :H].T is step-invariant -> precomputed once into S0.
  - trend/season head + residual sum in a tail phase.
"""
import sys

sys.path.insert(0, "/opt/trn_rl_repo")
import numpy as np

import concourse.bass as bass
import concourse.mybir as mybir
import concourse.tile as tile
from concourse import bacc
from concourse.bass import ds, ts
from concourse.bass_utils import run_bass_kernel_spmd

F16 = mybir.dt.float16
F32 = mybir.dt.float32
AF = mybir.ActivationFunctionType

BS, H, D, SEQ = 128, 1024, 128, 256
NCORES = 8
B = BS // NCORES          # 16 rows per core
KH = H // 128             # 8 k-tiles over hidden dim
G3 = 3 * H                # 3072 gate cols
NCH = G3 // 512           # 6 psum chunks per gate set (4 rz + 2 n)
EMBED_DIM = 265216
TS_OFF = 3 * H
TS_LEN = SEQ * (H // 2)   # 131072


def build_nc(n_steps=SEQ, unroll=15, static=False):
    nc = bacc.Bacc()

    d_embed = nc.declare_dram_parameter("embed", [B, EMBED_DIM], F32, isOutput=False)
    d_x0 = nc.declare_dram_parameter("x0", [B, D], F32, isOutput=False)
    d_wx0 = nc.declare_dram_parameter("wx0", [128, G3], F16, isOutput=False)
    d_wg0 = nc.declare_dram_parameter("wg0", [128, KH, G3], F16, isOutput=False)
    d_whh0 = nc.declare_dram_parameter("whh0", [128, KH, G3], F16, isOutput=False)
    d_wih1 = nc.declare_dram_parameter("wih1", [128, KH, G3], F16, isOutput=False)
    d_whh1 = nc.declare_dram_parameter("whh1", [128, KH, G3], F16, isOutput=False)
    d_fct = nc.declare_dram_parameter("fct", [128, KH, D], F16, isOutput=False)
    d_pred = nc.declare_dram_parameter("predt", [128, 4, D], F16, isOutput=False)
    d_s0b = nc.declare_dram_parameter("s0b", [G3], F16, isOutput=False)
    d_s1b = nc.declare_dram_parameter("s1b", [G3], F16, isOutput=False)
    d_bhh0n = nc.declare_dram_parameter("bhh0n", [H], F16, isOutput=False)
    d_bhh1n = nc.declare_dram_parameter("bhh1n", [H], F16, isOutput=False)
    d_fcb = nc.declare_dram_parameter("fcb", [D, 1], F32, isOutput=False)
    d_pb2 = nc.declare_dram_parameter("pb2", [D], F32, isOutput=False)
    d_id16 = nc.declare_dram_parameter("id16", [B, B], F16, isOutput=False)
    d_out = nc.declare_dram_parameter("out", [B, SEQ, D], F32, isOutput=True)

    # DRAM scratch (Tile tracks WAR deps on these)
    d_bc0 = nc.dram_tensor("bc0", [B, H], F16)
    d_bc1 = nc.dram_tensor("bc1", [B, H], F16)
    d_bg = nc.dram_tensor("bg", [B, H], F16)
    d_bx = nc.dram_tensor("bx", [B, D], F16)
    d_bts = nc.dram_tensor("bts", [128, 512], F16)

    def bcast(ap_1d, parts, n):
        return bass.AP(tensor=ap_1d.tensor, offset=ap_1d.offset,
                       ap=[[0, parts]] + list(ap_1d.ap))

    with tile.TileContext(nc) as tc:
        with (
            tc.tile_pool(name="persist", bufs=1) as pp,
            tc.tile_pool(name="tmp", bufs=2) as tp,
            tc.tile_pool(name="psum", bufs=8, space="PSUM") as qq,
        ):
            # ---------------- resident tiles ----------------
            s_wx0 = pp.tile([128, G3], F16)
            s_whh0 = pp.tile([128, KH, G3], F16)
            s_wih1 = pp.tile([128, KH, G3], F16)
            s_whh1 = pp.tile([128, KH, G3], F16)
            s_fct = pp.tile([128, KH, D], F16)
            s_pred = pp.tile([128, 4, D], F16)
            s_s0 = pp.tile([B, G3], F16)      # glob@Wg + biases (L0)
            s_s1 = pp.tile([B, G3], F16)      # L1 biases (broadcast)
            s_bhh0n = pp.tile([B, H], F16)
            s_bhh1n = pp.tile([B, H], F16)
            s_fcb = pp.tile([D, 1], F32)
            s_fcbrow = pp.tile([B, D], F32)
            s_pb2 = pp.tile([128, D], F32)
            s_h0 = pp.tile([B, H], F32)       # master states
            s_h1 = pp.tile([B, H], F32)
            s_h0t = pp.tile([128, KH, B], F16)
            s_h1t = pp.tile([128, KH, B], F16)
            s_xt = pp.tile([128, B], F16)
            s_trz = pp.tile([B, 2 * H], F16)  # rz pre-act, sigmoid'd in place
            s_nt = pp.tile([B, H], F32)
            s_dt = pp.tile([B, H], F32)
            s_ch = pp.tile([B, H], F16)
            s_xo = pp.tile([B, D], F32)
            s_id16 = pp.tile([B, B], F16)

            nc.sync.dma_start(out=s_wx0, in_=d_wx0[:, :])
            nc.sync.dma_start(out=s_whh0, in_=d_whh0[:, :, :])
            nc.sync.dma_start(out=s_wih1, in_=d_wih1[:, :, :])
            nc.sync.dma_start(out=s_whh1, in_=d_whh1[:, :, :])
            nc.sync.dma_start(out=s_fct, in_=d_fct[:, :, :])
            nc.sync.dma_start(out=s_pred, in_=d_pred[:, :, :])
            nc.gpsimd.dma_start(out=s_s0, in_=bcast(d_s0b[:], B, G3))
            nc.gpsimd.dma_start(out=s_s1, in_=bcast(d_s1b[:], B, G3))
            nc.gpsimd.dma_start(out=s_bhh0n, in_=bcast(d_bhh0n[:], B, H))
            nc.gpsimd.dma_start(out=s_bhh1n, in_=bcast(d_bhh1n[:], B, H))
            nc.sync.dma_start(out=s_fcb, in_=d_fcb[:, :])
            nc.gpsimd.dma_start(out=s_fcbrow, in_=bcast(d_fcb[:, 0], B, D))
            nc.gpsimd.dma_start(out=s_pb2, in_=bcast(d_pb2[:], 128, D))
            nc.sync.dma_start(out=s_id16, in_=d_id16[:, :])
            nc.sync.dma_start(out=s_h0, in_=d_embed[:, H:2 * H])
            nc.sync.dma_start(out=s_h1, in_=d_embed[:, 2 * H:3 * H])

            # initial transposed states
            nc.scalar.activation(s_ch, s_h0, AF.Copy)
            nc.sync.dma_start(out=d_bc0[:, :], in_=s_ch)
            nc.sync.dma_start_transpose(s_h0t[:], d_bc0[:, :])
            s_c1i = tp.tile([B, H], F16, tag="bigtmp")
            nc.scalar.activation(s_c1i, s_h1, AF.Copy)
            nc.sync.dma_start(out=d_bc1[:, :], in_=s_c1i)
            nc.sync.dma_start_transpose(s_h1t[:], d_bc1[:, :])

            # x0 -> xT
            s_x0f = tp.tile([B, D], F32, tag="bigtmp")
            nc.sync.dma_start(out=s_x0f, in_=d_x0[:, :])
            s_x0h = tp.tile([B, D], F16, tag="bigtmp")
            nc.scalar.activation(s_x0h, s_x0f, AF.Copy)
            nc.sync.dma_start(out=d_bx[:, :], in_=s_x0h)
            s_xtT = tp.tile([128, 1, B], F16, tag="bigtmp")
            nc.sync.dma_start_transpose(s_xtT[:], d_bx[:, :])
            nc.vector.tensor_copy(s_xt, s_xtT.rearrange("p o b -> p (o b)"))

            # glob -> globT; S0 = glob @ Wg0 + bias (bias pre-loaded into s_s0)
            s_gf = tp.tile([B, H], F32, tag="wg", bufs=1)
            nc.sync.dma_start(out=s_gf, in_=d_embed[:, 0:H])
            s_gh = tp.tile([B, H], F16, tag="bigtmp")
            nc.scalar.activation(s_gh, s_gf, AF.Copy)
            nc.sync.dma_start(out=d_bg[:, :], in_=s_gh)
            s_gT = tp.tile([128, KH, B], F16, tag="bigtmp")
            nc.sync.dma_start_transpose(s_gT[:], d_bg[:, :])
            pg = [qq.tile([B, 512], F32, tag="ps", name=f"pg{c}") for c in range(NCH)]
            for k in range(KH):
                wbuf = tp.tile([128, G3], F16, tag="wg", bufs=1)
                nc.sync.dma_start(out=wbuf, in_=d_wg0[:, k, :])
                for c in range(NCH):
                    nc.tensor.matmul(pg[c], s_gT[:, k, :], wbuf[:, ts(c, 512)],
                                     start=(k == 0), stop=(k == KH - 1))
            for c in range(NCH):
                nc.vector.tensor_add(s_s0[:, ts(c, 512)], pg[c], s_s0[:, ts(c, 512)])

            # ---------------- one recurrence step ----------------
            # Per-half-H gate math: consumes the 4 psum chunks of one half
            # (r, z from p_rz; i_n in p_ci; h_n in p_ch) and produces the
            # updated master-state half + its transposed fp16 k-tiles.
            def gate_math_half(c, p_rz, p_ci, p_ch, s_s, s_bhhn, s_h, s_ht, d_bc):
                sl = ts(c, 512)
                zsl = slice(H + c * 512, H + (c + 1) * 512)
                nc.scalar.activation(s_trz[:, sl], p_rz[0], AF.Sigmoid)
                nc.scalar.activation(s_trz[:, zsl], p_rz[1], AF.Sigmoid)
                nc.vector.tensor_add(p_ch, p_ch, s_bhhn[:, sl])
                nc.vector.tensor_add(p_ci, p_ci, s_s[:, 2 * H + c * 512:2 * H + (c + 1) * 512])
                nc.vector.tensor_mul(s_nt[:, sl], s_trz[:, sl], p_ch)
                nc.vector.tensor_add(s_nt[:, sl], p_ci, s_nt[:, sl])
                nc.scalar.activation(s_nt[:, sl], s_nt[:, sl], AF.Tanh)
                nc.vector.tensor_sub(s_dt[:, sl], s_h[:, sl], s_nt[:, sl])
                nc.vector.tensor_mul(s_dt[:, sl], s_trz[:, zsl], s_dt[:, sl])
                nc.vector.tensor_add(s_h[:, sl], s_nt[:, sl], s_dt[:, sl])
                nc.scalar.activation(s_ch[:, sl], s_h[:, sl], AF.Copy)
                nc.sync.dma_start(out=d_bc[:, c * 512:(c + 1) * 512], in_=s_ch[:, sl])
                nc.sync.dma_start_transpose(s_ht[:, 4 * c:4 * c + 4, :],
                                            d_bc[:, c * 512:(c + 1) * 512])

            def fc_and_out(t_expr):
                # transposed fc -> xT (recurrence operand)
                pfcT = qq.tile([D, B], F32, tag="ps")
                for k in range(KH):
                    nc.tensor.matmul(pfcT, s_fct[:, k, :], s_h1t[:, k, :],
                                     start=(k == 0), stop=(k == KH - 1))
                nc.scalar.activation(s_xt, pfcT, AF.Sigmoid, bias=s_fcb[:, :])
                # batch-major fc -> output row
                pfcA = qq.tile([B, D], F32, tag="ps")
                for k in range(KH):
                    nc.tensor.matmul(pfcA, s_h1t[:, k, :], s_fct[:, k, :],
                                     start=(k == 0), stop=(k == KH - 1))
                nc.vector.tensor_add(s_xo, pfcA, s_fcbrow)
                nc.scalar.activation(s_xo, s_xo, AF.Sigmoid)
                nc.sync.dma_start(out=d_out[:, ds(t_expr, 1), :],
                                  in_=s_xo.rearrange("b d -> b () d"))

            def l0_half(h, tag):
                """gh0 matmuls for one half of H -> 4 fresh psum chunks.
                Chunk-major so each psum bank is claimed only when needed,
                matching the order the previous consumer releases banks."""
                rsl = ts(h, 512)                                  # r cols
                zsl = slice(H + h * 512, H + (h + 1) * 512)       # z cols
                nsl = slice(2 * H + h * 512, 2 * H + (h + 1) * 512)
                p_rz = [None, None]
                p_rz[0] = qq.tile([B, 512], F32, tag="ps", name=f"{tag}rz0")
                for k in range(KH):
                    nc.tensor.matmul(p_rz[0], s_h0t[:, k, :], s_whh0[:, k, rsl],
                                     start=(k == 0), stop=False)
                p_rz[1] = qq.tile([B, 512], F32, tag="ps", name=f"{tag}rz1")
                for k in range(KH):
                    nc.tensor.matmul(p_rz[1], s_h0t[:, k, :], s_whh0[:, k, zsl],
                                     start=(k == 0), stop=False)
                p_ch = qq.tile([B, 512], F32, tag="ps", name=f"{tag}ch")
                for k in range(KH):
                    nc.tensor.matmul(p_ch, s_h0t[:, k, :], s_whh0[:, k, nsl],
                                     start=(k == 0), stop=(k == KH - 1))
                return p_rz, p_ch

            def gi0x_half(h, p_rz, tag):
                rsl = ts(h, 512)
                zsl = slice(H + h * 512, H + (h + 1) * 512)
                nsl = slice(2 * H + h * 512, 2 * H + (h + 1) * 512)
                nc.tensor.matmul(p_rz[0], s_xt, s_wx0[:, rsl], start=False, stop=False)
                nc.tensor.matmul(p_rz[0], s_id16, s_s0[:, rsl], start=False, stop=True)
                nc.tensor.matmul(p_rz[1], s_xt, s_wx0[:, zsl], start=False, stop=False)
                nc.tensor.matmul(p_rz[1], s_id16, s_s0[:, zsl], start=False, stop=True)
                p_ci = qq.tile([B, 512], F32, tag="ps", name=f"{tag}ci")
                nc.tensor.matmul(p_ci, s_xt, s_wx0[:, nsl], start=True, stop=True)
                return p_ci

            def l1_half(h, tag):
                """gh1 + gi1 matmuls for one half of H -> 4 fresh psum chunks."""
                rsl = ts(h, 512)
                zsl = slice(H + h * 512, H + (h + 1) * 512)
                nsl = slice(2 * H + h * 512, 2 * H + (h + 1) * 512)
                p_rz = [None, None]
                p_rz[0] = qq.tile([B, 512], F32, tag="ps", name=f"{tag}rz0")
                for k in range(KH):
                    nc.tensor.matmul(p_rz[0], s_h1t[:, k, :], s_whh1[:, k, rsl],
                                     start=(k == 0), stop=False)
                p_rz[1] = qq.tile([B, 512], F32, tag="ps", name=f"{tag}rz1")
                for k in range(KH):
                    nc.tensor.matmul(p_rz[1], s_h1t[:, k, :], s_whh1[:, k, zsl],
                                     start=(k == 0), stop=False)
                p_ch = qq.tile([B, 512], F32, tag="ps", name=f"{tag}ch")
                for k in range(KH):
                    nc.tensor.matmul(p_ch, s_h1t[:, k, :], s_whh1[:, k, nsl],
                                     start=(k == 0), stop=(k == KH - 1))
                # gi1 (contracts this step's c0; h0t tiles come from B-h0/B-h1)
                for k in range(KH):
                    nc.tensor.matmul(p_rz[0], s_h0t[:, k, :], s_wih1[:, k, rsl],
                                     start=False, stop=False)
                nc.tensor.matmul(p_rz[0], s_id16, s_s1[:, rsl], start=False, stop=True)
                for k in range(KH):
                    nc.tensor.matmul(p_rz[1], s_h0t[:, k, :], s_wih1[:, k, zsl],
                                     start=False, stop=False)
                nc.tensor.matmul(p_rz[1], s_id16, s_s1[:, zsl], start=False, stop=True)
                p_ci = qq.tile([B, 512], F32, tag="ps", name=f"{tag}ci")
                for k in range(KH):
                    nc.tensor.matmul(p_ci, s_h0t[:, k, :], s_wih1[:, k, nsl],
                                     start=(k == 0), stop=(k == KH - 1))
                return p_rz, p_ci, p_ch

            def step(t_expr, first):
                # L0 half 0 (covers the tail of previous step's E-h1 chain)
                rz0a, ch0a = l0_half(0, "a")
                # previous step's fc + output (produces s_xt for this step)
                if not first:
                    fc_and_out(t_expr - 1)
                # L0 half 1 (covers the fc->sigmoid->xT latency)
                rz0b, ch0b = l0_half(1, "b")
                ci0a = gi0x_half(0, rz0a, "a")
                gate_math_half(0, rz0a, ci0a, ch0a, s_s0, s_bhh0n, s_h0, s_h0t, d_bc0)
                ci0b = gi0x_half(1, rz0b, "b")
                gate_math_half(1, rz0b, ci0b, ch0b, s_s0, s_bhh0n, s_h0, s_h0t, d_bc0)
                # L1: both halves' matmuls first (they contract the FULL old c1,
                # so no E gate math may overwrite s_h1t before these are emitted)
                rz1a, ci1a, ch1a = l1_half(0, "c")
                rz1b, ci1b, ch1b = l1_half(1, "d")
                gate_math_half(0, rz1a, ci1a, ch1a, s_s1, s_bhh1n, s_h1, s_h1t, d_bc1)
                gate_math_half(1, rz1b, ci1b, ch1b, s_s1, s_bhh1n, s_h1, s_h1t, d_bc1)

            if static:
                for t in range(n_steps):
                    step(t, t == 0)
            else:
                step(0, True)
                while (n_steps - 1) % unroll != 0:
                    unroll -= 1
                with tc.For_i(1, n_steps, unroll,
                              hint_engines=(mybir.EngineType.PE,)) as iv:
                    for j in range(unroll):
                        step(iv + j, False)
            fc_and_out(n_steps - 1)

            # ---------------- tail: trend/season + residual ----------------
            for b in range(B):
                for si in range(2):
                    base = TS_OFF + si * 128 * 512
                    ps_o = qq.tile([128, D], F32, tag="ps")
                    for which in range(2):  # 0=trend 1=season
                        off = base + which * TS_LEN
                        src = d_embed[b:b + 1, off:off + 65536].rearrange(
                            "o (s f) -> (o s) f", f=512)
                        t_f = tp.tile([128, 512], F32, tag="tsf")
                        nc.sync.dma_start(out=t_f, in_=src)
                        t_h = tp.tile([128, 512], F16, tag="bigtmp")
                        nc.scalar.activation(t_h, t_f, AF.Copy)
                        nc.sync.dma_start(out=d_bts[:, :], in_=t_h)
                        t_T = tp.tile([128, 4, 128], F16, tag="bigtmp")
                        nc.sync.dma_start_transpose(t_T[:], d_bts[:, :])
                        for jj in range(4):
                            nc.tensor.matmul(ps_o, t_T[:, jj, :], s_pred[:, jj, :],
                                             start=(which == 0 and jj == 0),
                                             stop=(which == 1 and jj == 3))
                    r_c = tp.tile([128, D], F32, tag="bigtmp")
                    nc.sync.dma_start(out=r_c, in_=d_out[b, si * 128:(si + 1) * 128, :])
                    nc.vector.tensor_add(r_c, ps_o, r_c)
                    nc.vector.tensor_add(r_c, r_c, s_pb2)
                    nc.sync.dma_start(out=d_out[b, si * 128:(si + 1) * 128, :], in_=r_c)

    nc.compile()
    return nc


def _prep_weights(W_ih0, W_hh0, b_ih0, b_hh0, W_ih1, W_hh1, b_ih1, b_hh1,
                  fc_W, fc_b, pred_W, pred_b):
    f16 = np.float16

    def karr(WT):  # [K, N] -> [128, K/128, N]
        K, N = WT.shape
        return np.ascontiguousarray(
            WT.reshape(K // 128, 128, N).transpose(1, 0, 2)).astype(f16)

    return dict(
        wx0=np.ascontiguousarray(W_ih0[:, H:H + D].T).astype(f16),
        wg0=karr(W_ih0[:, :H].T),
        whh0=karr(W_hh0.T),
        wih1=karr(W_ih1.T),
        whh1=karr(W_hh1.T),
        fct=karr(fc_W.T),
        predt=np.ascontiguousarray(
            pred_W.T.reshape(4, 128, D).transpose(1, 0, 2)).astype(f16),
        s0b=np.concatenate([(b_ih0 + b_hh0)[:2 * H], b_ih0[2 * H:]]).astype(f16),
        s1b=np.concatenate([(b_ih1 + b_hh1)[:2 * H], b_ih1[2 * H:]]).astype(f16),
        bhh0n=b_hh0[2 * H:].astype(f16),
        bhh1n=b_hh1[2 * H:].astype(f16),
        fcb=np.ascontiguousarray(fc_b.reshape(D, 1)).astype(np.float32),
        id16=np.eye(B, dtype=np.float16),
        pb2=(2.0 * pred_b).astype(np.float32),
    )


_NC_CACHE = {}


def kernel(embed, dynamics, W_ih0, W_hh0, b_ih0, b_hh0,
           W_ih1, W_hh1, b_ih1, b_hh1, fc_W, fc_b, pred_W, pred_b, seq_len,
           _n_steps=SEQ, _static=False, _trace=False):
    embed = np.asarray(embed, dtype=np.float32)
    dynamics = np.asarray(dynamics, dtype=np.float32)
    wd = _prep_weights(np.asarray(W_ih0, np.float32), np.asarray(W_hh0, np.float32),
                       np.asarray(b_ih0, np.float32), np.asarray(b_hh0, np.float32),
                       np.asarray(W_ih1, np.float32), np.asarray(W_hh1, np.float32),
                       np.asarray(b_ih1, np.float32), np.asarray(b_hh1, np.float32),
                       np.asarray(fc_W, np.float32), np.asarray(fc_b, np.float32),
                       np.asarray(pred_W, np.float32), np.asarray(pred_b, np.float32))

    key = (_n_steps, _static)
    if key not in _NC_CACHE:
        _NC_CACHE[key] = build_nc(n_steps=_n_steps, static=_static)
    nc = _NC_CACHE[key]

    in_maps = []
    for c in range(NCORES):
        m = dict(wd)
        m["embed"] = np.ascontiguousarray(embed[c * B:(c + 1) * B])
        m["x0"] = np.ascontiguousarray(dynamics[c * B:(c + 1) * B, 0, :])
        in_maps.append(m)

    res = run_bass_kernel_spmd(nc, in_maps, list(range(NCORES)), trace=False)
    out = np.concatenate([res.results[c]["out"] for c in range(NCORES)], axis=0)
    if _trace:
        kernel.last_exec_time_ns = _bench_exec(nc, in_maps)
    return out


def _bench_exec(nc, in_maps, n_timed=7):
    """Median wall time of the sharded NEFF execution with device-resident
    inputs (the NTFF profiling hook is unavailable under this axon client,
    so time repeated executions instead)."""
    import time

    import jax
    import jax.numpy as jnp
    from jax.sharding import Mesh, NamedSharding, PartitionSpec
    from jax.experimental.shard_map import shard_map

    from concourse import bass2jax, mybir as _mb

    bass2jax.install_neuronx_cc_hook()
    n_cores = len(in_maps)
    partition_name = (nc.partition_id_tensor.name if nc.partition_id_tensor else None)
    in_names, out_names, out_avals, zero_outs = [], [], [], []
    for alloc in nc.m.functions[0].allocations:
        if not isinstance(alloc, _mb.MemoryLocationSet):
            continue
        name = alloc.memorylocations[0].name
        if alloc.kind == "ExternalInput":
            if name != partition_name:
                in_names.append(name)
        elif alloc.kind == "ExternalOutput":
            out_names.append(name)
            shape = tuple(alloc.tensor_shape)
            dtype = _mb.dt.np(alloc.dtype)
            out_avals.append(jax.core.ShapedArray(shape, dtype))
            zero_outs.append(np.zeros(shape, dtype))
    n_params = len(in_names)
    all_names = list(in_names) + out_names
    if partition_name is not None:
        all_names.append(partition_name)

    def _body(*args):
        operands = list(args)
        if partition_name is not None:
            operands.append(bass2jax.partition_id_tensor())
        return tuple(bass2jax._bass_exec_p.bind(
            *operands,
            out_avals=tuple(out_avals),
            in_names=tuple(all_names),
            out_names=tuple(out_names),
            lowering_input_output_aliases=(),
            sim_require_finite=False,
            sim_require_nnan=False,
            nc=nc,
        ))

    devices = jax.devices()[:n_cores]
    mesh = Mesh(np.asarray(devices), ("core",))
    spec = PartitionSpec("core")
    fn = jax.jit(shard_map(
        _body, mesh=mesh,
        in_specs=(spec,) * (n_params + len(out_names)),
        out_specs=(spec,) * len(out_names), check_rep=False))
    sh = NamedSharding(mesh, spec)
    dev_in = [jax.device_put(
        np.concatenate([np.asarray(in_maps[c][nm]) for c in range(n_cores)], axis=0), sh)
        for nm in in_names]
    dev_zo = [jax.device_put(np.concatenate([z] * n_cores, axis=0), sh) for z in zero_outs]
    r = fn(*dev_in, *dev_zo)
    jax.block_until_ready(r)
    times = []
    for _ in range(n_timed):
        t0 = time.perf_counter()
        r = fn(*dev_in, *dev_zo)
        jax.block_until_ready(r)
        times.append(time.perf_counter() - t0)
    return int(min(times) * 1e9)

